# revision 15
# baseline (speedup 1.0000x reference)
"""Trainium2 Bass kernel for nn_DMM (deep Markov model: reverse LSTM +
sequential inference recursion + parallel generation).

Contract: kernel(**inputs) takes FULL unsharded inputs (numpy), returns the
FULL output tuple (y, z_mean, z_logvar, z_mean_p, z_logvar_p, z_out).
Internally shards batch (256) across 8 NeuronCores, data-parallel with
replicated weights.

Device layouts are "transposed": feature dims live on SBUF partitions, batch
and/or time on the free dim, so the serial recurrences never need per-step
transposes and elementwise ops use all 128 lanes.
"""

import ml_dtypes
import numpy as np

import concourse.bass as bass
import concourse.mybir as mybir
import concourse.tile as tile
from concourse import bacc
from concourse import bass_utils

AF = mybir.ActivationFunctionType
OP = mybir.AluOpType
F32 = mybir.dt.float32
F32R = mybir.dt.float32r
BF16 = mybir.dt.bfloat16


def _r(ap):
    """View an fp32 AP as float32r for full-rate PE streaming (N>=256)."""
    return ap.bitcast(F32R)

B, T, XD, ZD, H = 256, 300, 513, 16, 128
NCORES = 8
BS = B // NCORES  # 32 sequences per core
NG = 4            # gates
KC = 5            # k-chunks of XD=513 (4*128 + 1)
TCH = 30          # time-chunk for stat staging DMA
PHASES = (1, 1, 1, 1)  # debug: enable xproj/lstm/inf/gen
# gate order used on-device: [i, f, o, g]  (PyTorch order in weights: i,f,g,o)
GPERM = np.concatenate([np.arange(0, 128), np.arange(128, 256),
                        np.arange(384, 512), np.arange(256, 384)])


def _emit(nc, tc, ins, outs, Tn, Bs, ctx):
    TB = Tn * Bs
    nch = (Tn + TCH - 1) // TCH

    # ---------------- pools ----------------
    wpool = ctx.enter_context(tc.tile_pool(name="weights", bufs=1))
    zb_cm = tc.tile_pool(name="zb", bufs=1)
    zb_pool = zb_cm.__enter__()
    ZB = zb_pool.tile([16, Bs, Tn], F32)

    # ---------------- load weights ----------------
    def wtile(pool, name, shape, dt=F32):
        t = pool.tile(list(shape), dt, tag=name)
        nc.sync.dma_start(out=t, in_=ins[name])
        return t

    bG = wtile(wpool, "bG", (128, NG))            # combined gate bias [i,f,o,g]
    Wh = wtile(wpool, "Wh", (128, NG * H))        # (2*W_hh[perm]).T
    Wzz = wtile(wpool, "Wzz", (ZD, H))
    bzz = wtile(wpool, "bzz", (128, 1))
    Wim = wtile(wpool, "Wim", (128, ZD))          # W_im.T
    Wil = wtile(wpool, "Wil", (128, ZD))          # W_il.T
    Wimh = wtile(wpool, "Wimh", (128, ZD))        # (0.5*W_im).T
    Wilh = wtile(wpool, "Wilh", (128, ZD))        # (0.5*W_il).T
    IdM = wtile(wpool, "IdM", (128, 128), BF16)   # identity (Xp psum inject)
    bim = wtile(wpool, "bim", (ZD, 1))
    bil = wtile(wpool, "bil", (ZD, 1))
    bil2 = wtile(wpool, "bil2", (ZD, 1))          # 0.5*b_il
    zw = {}
    for nm in ("g1", "g2", "p1", "p2", "pm", "pl"):
        zw["W" + nm] = wtile(wpool, "W" + nm, (ZD, ZD), BF16)
        zw["b" + nm] = wtile(wpool, "b" + nm, (ZD, 1))
    Wzx1 = wtile(wpool, "Wzx1", (ZD, 128), BF16)
    bzx1 = wtile(wpool, "bzx1", (128, 1))
    Wzx2 = wtile(wpool, "Wzx2", (128, 128), BF16)
    bzx2 = wtile(wpool, "bzx2", (128, 1))
    Wgy = wtile(wpool, "Wgy", (128, XD), BF16)    # W_gy.T
    bgy = wtile(wpool, "bgy", (128, KC))          # per-chunk bias columns

    # ---------------- persistent state (phases 1-3) ----------------
    gh_cm = tc.tile_pool(name="gh", bufs=1)
    gh_pool = gh_cm.__enter__()
    # GH: h/2 for every t (W_hh pre-doubled on host to compensate)
    GH = gh_pool.tile([128, Tn, Bs], F32)

    xp_cm = tc.tile_pool(name="xp", bufs=1)
    xp_pool = xp_cm.__enter__()
    # XP: x-projection + bias, transposed layout, bf16
    XP = xp_pool.tile([128, Tn, NG, Bs], BF16)

    zero16 = wpool.tile([16, Bs], F32, tag="zero16")
    nc.vector.memset(zero16, 0.0)
    zero128 = wpool.tile([128, Bs], F32, tag="zero128")
    nc.vector.memset(zero128, 0.0)

    # ---------------- phase 1: x projection ----------------
    x_cm = tc.tile_pool(name="xio", bufs=4)
    xio = x_cm.__enter__()
    wx_cm = tc.tile_pool(name="wx", bufs=1)
    wx_pool = wx_cm.__enter__()
    ppx_cm = tc.tile_pool(name="ppx", bufs=8, space="PSUM")
    ppool = ppx_cm.__enter__()
    Wx = wtile(wx_pool, "Wx", (128, KC, NG * H), F32R)  # packed W_ih.T

    xb = ins["xb"]  # (Bs, XD, Tn)
    BGRP = 4
    for b0 in (range(0, Bs, BGRP) if PHASES[0] else []):
        xts = []
        for bi in range(BGRP):
            xt = xio.tile([128, KC, Tn], F32R, tag="xt")
            nc.sync.dma_start(
                out=xt[:, 0:4, :],
                in_=xb[b0 + bi, 0:512, :].rearrange("(k p) t -> p k t", p=128),
            )
            nc.sync.dma_start(out=xt[0:1, 4, :], in_=xb[b0 + bi, 512:513, :])
            xts.append(xt)
        for g in range(NG):
            pss = [ppool.tile([128, Tn], F32, tag="xproj", name=f"psx{g}_{bi}")
                   for bi in range(BGRP)]
            for k in range(KC):
                kk = 128 if k < 4 else 1
                for bi in range(BGRP):
                    nc.tensor.matmul(
                        pss[bi],
                        Wx[0:kk, k, g * H:(g + 1) * H],
                        xts[bi][0:kk, k, :],
                        start=(k == 0),
                        stop=(k == KC - 1),
                    )
            for bi in range(BGRP):
                # psum -> XP (+gate bias), bf16
                nc.vector.tensor_scalar_add(XP[:, :, g, b0 + bi], pss[bi],
                                            bG[:, g:g + 1])
    ppx_cm.__exit__(None, None, None)
    wx_cm.__exit__(None, None, None)
    x_cm.__exit__(None, None, None)

    # ---------------- phase 2: LSTM over reversed time ----------------
    lstm_cm = tc.tile_pool(name="lstm", bufs=3)
    lstm = lstm_cm.__enter__()
    ppl_cm = tc.tile_pool(name="ppl", bufs=4, space="PSUM")
    ppool = ppl_cm.__enter__()
    cstate = gh_pool.tile([128, Bs], F32, tag="cstate")
    nc.vector.memset(cstate, 0.0)
    if not PHASES[1]:
        nc.vector.memset(GH, 0.0)
    # Per step: psum G = IdM.T @ XP[t] (bias+x-proj inject) + W_hh MMs.
    # Gate-g pre-activations are host-doubled so tanh(g) = 2*sigmoid(2g)-1
    # and ONE sigmoid covers all four gates.
    for r in (range(Tn) if PHASES[1] else []):
        t = Tn - 1 - r           # original time index
        h_prev = zero128 if r == 0 else GH[:, t + 1, :]
        ps = ppool.tile([128, NG, Bs], F32, tag="lstm_g")
        nc.tensor.matmul(ps, IdM, XP[:, t, :, :], start=True, stop=False)
        for g in range(NG):
            nc.tensor.matmul(ps[:, g, :], Wh[:, g * H:(g + 1) * H], h_prev,
                             start=False, stop=(g == NG - 1))
        S = lstm.tile([128, NG, Bs], F32, tag="S")
        nc.scalar.activation(S, ps, AF.Sigmoid)
        t1 = lstm.tile([128, Bs], F32, tag="t1")
        nc.vector.tensor_mul(t1, S[:, 0, :], S[:, 3, :])
        nc.vector.tensor_mul(cstate, cstate, S[:, 1, :])
        c2 = lstm.tile([128, Bs], F32, tag="c2")
        nc.vector.scalar_tensor_tensor(c2, t1, 2.0, S[:, 0, :],
                                       op0=OP.mult, op1=OP.subtract)
        nc.vector.tensor_add(cstate, cstate, c2)
        TC = lstm.tile([128, Bs], F32, tag="TC")
        nc.scalar.activation(TC, cstate, AF.Tanh)
        # GH[t] = 0.5 * sigmoid(o) * tanh(c)
        nc.vector.scalar_tensor_tensor(GH[:, t, :], S[:, 2, :], 0.5, TC,
                                       op0=OP.mult, op1=OP.mult)
    ppl_cm.__exit__(None, None, None)
    lstm_cm.__exit__(None, None, None)
    xp_cm.__exit__(None, None, None)

    # ---------------- phase 3: inference recursion ----------------
    # z lives in SBUF (ZB) for the whole run; mean/logvar go out via
    # per-chunk staging DMAs.
    stage_cm = tc.tile_pool(name="stage", bufs=2)
    stage = stage_cm.__enter__()
    inf_cm = tc.tile_pool(name="inf", bufs=3)
    inf = inf_cm.__enter__()
    ppi_cm = tc.tile_pool(name="ppi", bufs=2, space="PSUM")
    ppool = ppi_cm.__enter__()
    z_prev = zero16
    if not PHASES[2]:
        nc.vector.memset(ZB, 0.0)
    for ci in (range(nch) if PHASES[2] else []):
        t0 = ci * TCH
        tw = min(TCH, Tn - t0)
        eps_c = stage.tile([16, Bs, TCH], F32, tag="eps_c")
        nc.sync.dma_start(out=eps_c[:, :, 0:tw],
                          in_=ins["epsT"][:, :, t0:t0 + tw])
        mean_c = stage.tile([16, Bs, TCH], F32, tag="mean_c")
        lv_c = stage.tile([16, Bs, TCH], F32, tag="lv_c")
        for tm in range(tw):
            t = t0 + tm
            ps1 = ppool.tile([128, Bs], F32, tag="inf1")
            nc.tensor.matmul(ps1, Wzz, z_prev, start=True, stop=True)
            TH = inf.tile([128, Bs], F32, tag="TH")
            nc.scalar.activation(TH, ps1, AF.Tanh, bias=bzz)
            # mean/logvar = W @ (0.5*tanh + h/2): fused as two accumulating
            # matmuls each; gt never materializes.
            psl = ppool.tile([16, Bs], F32, tag="inf3")
            nc.tensor.matmul(psl, Wilh, TH, start=True, stop=False)
            nc.tensor.matmul(psl, Wil, GH[:, t, :], start=False, stop=True)
            psm = ppool.tile([16, Bs], F32, tag="inf2")
            nc.tensor.matmul(psm, Wimh, TH, start=True, stop=False)
            nc.tensor.matmul(psm, Wim, GH[:, t, :], start=False, stop=True)
            E = inf.tile([16, Bs], F32, tag="E")
            nc.scalar.activation(E, psl, AF.Exp, bias=bil2, scale=0.5)
            nc.vector.tensor_scalar_add(mean_c[:, :, tm], psm, bim)
            nc.scalar.activation(lv_c[:, :, tm], psl, AF.Identity, bias=bil)
            t2 = inf.tile([16, Bs], F32, tag="t2")
            nc.vector.tensor_mul(t2, E, eps_c[:, :, tm])
            nc.vector.tensor_add(ZB[:, :, t], mean_c[:, :, tm], t2)
            z_prev = ZB[:, :, t]
        nc.sync.dma_start(out=outs["zmean"][:, :, t0:t0 + tw],
                          in_=mean_c[:, :, 0:tw])
        nc.sync.dma_start(out=outs["zlogvar"][:, :, t0:t0 + tw],
                          in_=lv_c[:, :, 0:tw])
    ppi_cm.__exit__(None, None, None)
    inf_cm.__exit__(None, None, None)
    stage_cm.__exit__(None, None, None)
    gh_cm.__exit__(None, None, None)

    # ---------------- phase 4: generation ----------------
    gen_cm = tc.tile_pool(name="gen", bufs=1)
    gen = gen_cm.__enter__()
    genw_cm = tc.tile_pool(name="genw", bufs=3)
    genw = genw_cm.__enter__()
    ppz_cm = tc.tile_pool(name="ppz", bufs=2, space="PSUM")
    ppool = ppz_cm.__enter__()

    ZBF = ZB.rearrange("p b t -> p (b t)")   # z flat, b-major
    nc.sync.dma_start(out=outs["zout"], in_=ZB)
    z_bf = gen.tile([16, TB], BF16)       # bf16 z for full-rate matmuls
    nc.vector.tensor_copy(z_bf, ZBF)
    SPIN = gen.tile([16, TB], F32)        # softplus input staging

    # pass 1: z-MLPs (sigmoid table set)
    NBLK = 512
    nblocks = (TB + NBLK - 1) // NBLK
    mp_flat = outs["zmeanp"].rearrange("p b t -> p (b t)")
    for i in (range(nblocks) if PHASES[3] else []):
        lo = i * NBLK
        w = min(NBLK, TB - lo)
        zb = z_bf[:, lo:lo + w]
        pg1 = ppool.tile([16, NBLK], F32, tag="zmlp")
        nc.tensor.matmul(pg1[:, 0:w], zw["Wg1"], zb, start=True, stop=True)
        r1 = genw.tile([16, NBLK], BF16, tag="r1")
        nc.vector.tensor_scalar(r1[:, 0:w], pg1[:, 0:w], zw["bg1"], 0.0,
                                op0=OP.add, op1=OP.max)
        pg2 = ppool.tile([16, NBLK], F32, tag="zmlp2")
        nc.tensor.matmul(pg2[:, 0:w], zw["Wg2"], r1[:, 0:w],
                         start=True, stop=True)
        gate = genw.tile([16, NBLK], F32, tag="gate")
        nc.scalar.activation(gate[:, 0:w], pg2[:, 0:w], AF.Sigmoid,
                             bias=zw["bg2"])
        pp1 = ppool.tile([16, NBLK], F32, tag="zmlp")
        nc.tensor.matmul(pp1[:, 0:w], zw["Wp1"], zb, start=True, stop=True)
        r2 = genw.tile([16, NBLK], BF16, tag="r2")
        nc.vector.tensor_scalar(r2[:, 0:w], pp1[:, 0:w], zw["bp1"], 0.0,
                                op0=OP.add, op1=OP.max)
        pp2 = ppool.tile([16, NBLK], F32, tag="zmlp2")
        nc.tensor.matmul(pp2[:, 0:w], zw["Wp2"], r2[:, 0:w],
                         start=True, stop=True)
        zprop = genw.tile([16, NBLK], F32, tag="zprop")
        nc.vector.tensor_scalar_add(zprop[:, 0:w], pp2[:, 0:w], zw["bp2"])
        ppm = ppool.tile([16, NBLK], F32, tag="zmlp")
        nc.tensor.matmul(ppm[:, 0:w], zw["Wpm"], zb, start=True, stop=True)
        pm = genw.tile([16, NBLK], F32, tag="pm")
        nc.vector.tensor_scalar_add(pm[:, 0:w], ppm[:, 0:w], zw["bpm"])
        # mean_p = pm + gate*(zprop - pm)
        d = genw.tile([16, NBLK], F32, tag="d")
        nc.vector.tensor_sub(d[:, 0:w], zprop[:, 0:w], pm[:, 0:w])
        t3 = genw.tile([16, NBLK], F32, tag="t3")
        nc.vector.tensor_mul(t3[:, 0:w], gate[:, 0:w], d[:, 0:w])
        mp_o = genw.tile([16, NBLK], F32, tag="mp_o")
        nc.vector.tensor_add(mp_o[:, 0:w], pm[:, 0:w], t3[:, 0:w])
        nc.sync.dma_start(out=mp_flat[:, lo:lo + w], in_=mp_o[:, 0:w])
        # logvar_p input: relu(zprop) @ Wpl.T + bpl
        rp = genw.tile([16, NBLK], BF16, tag="rp")
        nc.vector.tensor_scalar_max(rp[:, 0:w], zprop[:, 0:w], 0.0)
        ppl = ppool.tile([16, NBLK], F32, tag="zmlp2")
        nc.tensor.matmul(ppl[:, 0:w], zw["Wpl"], rp[:, 0:w],
                         start=True, stop=True)
        nc.vector.tensor_scalar_add(SPIN[:, lo:lo + w], ppl[:, 0:w],
                                    zw["bpl"])

    # pass 2: logvar_p = ln(softplus(v)) = ln(ln(1+exp(v))); Exp+Ln share
    # the natural_log_exp_and_others table set (no Softplus table on gen3)
    if not PHASES[3]:
        nc.vector.memset(SPIN, 0.0)
    nc.scalar.activation(SPIN, SPIN, AF.Exp)
    nc.vector.tensor_scalar_add(SPIN, SPIN, 1.0)
    nc.scalar.activation(SPIN, SPIN, AF.Ln)
    nc.scalar.activation(SPIN, SPIN, AF.Ln)
    nc.sync.dma_start(out=outs["zlogvarp"].rearrange("p b t -> p (b t)"),
                      in_=SPIN)

    ppz_cm.__exit__(None, None, None)
    genw_cm.__exit__(None, None, None)
    ppg_cm = tc.tile_pool(name="ppg", bufs=6, space="PSUM")
    ppool = ppg_cm.__enter__()

    # pass 3: h1/h2/y (exp table set: tanh + exp)
    gen2_cm = tc.tile_pool(name="gen2", bufs=1)
    gen2 = gen2_cm.__enter__()
    h1 = gen2.tile([128, TB], BF16)
    h2 = gen2.tile([128, TB], BF16)
    for b in range(Bs):
        ps = ppool.tile([128, Tn], F32, tag="gmm")
        nc.tensor.matmul(ps, Wzx1, z_bf[:, b * Tn:(b + 1) * Tn],
                         start=True, stop=True)
        nc.scalar.activation(h1[:, b * Tn:(b + 1) * Tn], ps, AF.Tanh,
                             bias=bzx1)
    for b in range(Bs):
        ps = ppool.tile([128, Tn], F32, tag="gmm")
        nc.tensor.matmul(ps, Wzx2, h1[:, b * Tn:(b + 1) * Tn],
                         start=True, stop=True)
        nc.scalar.activation(h2[:, b * Tn:(b + 1) * Tn], ps, AF.Tanh,
                             bias=bzx2)
    yout = outs["y"]
    ystage_cm = tc.tile_pool(name="ystage", bufs=4)
    ystage = ystage_cm.__enter__()
    for c in range(KC):
        pc = 128 if c < 4 else 1
        for b in range(Bs):
            ps = ppool.tile([128, Tn], F32, tag="gmm")
            nc.tensor.matmul(ps[0:pc, :], Wgy[:, c * 128:c * 128 + pc],
                             h2[:, b * Tn:(b + 1) * Tn],
                             start=True, stop=True)
            ys = ystage.tile([128, Tn], F32, tag="ys")
            nc.scalar.activation(ys[0:pc, :], ps[0:pc, :], AF.Exp,
                                 bias=bgy[0:pc, c:c + 1])
            nc.sync.dma_start(out=yout[b, c * 128:c * 128 + pc, :],
                              in_=ys[0:pc, :])
    ystage_cm.__exit__(None, None, None)
    ppg_cm.__exit__(None, None, None)
    gen2_cm.__exit__(None, None, None)
    gen_cm.__exit__(None, None, None)
    zb_cm.__exit__(None, None, None)


def _build_core_kernel(nc, tc, ins, outs, Tn=T, Bs=BS):
    from contextlib import ExitStack
    with ExitStack() as ctx:
        _emit(nc, tc, ins, outs, Tn, Bs, ctx)


def _prep_weights(inputs):
    """Host-side packing of the (tiny, replicated) weights."""
    f = lambda a: np.ascontiguousarray(a, dtype=np.float32)
    W_ih = np.asarray(inputs["W_ih"], np.float32)
    W_hh = np.asarray(inputs["W_hh"], np.float32)
    b_ih = np.asarray(inputs["b_ih"], np.float32)
    b_hh = np.asarray(inputs["b_hh"], np.float32)

    # gate-g (4th block after perm) pre-activations doubled: tanh(g) =
    # 2*sigmoid(2g) - 1 lets one sigmoid cover all gates
    gscale = np.concatenate([np.ones(3 * H, np.float32),
                             np.full(H, 2.0, np.float32)])
    WxT = (W_ih[GPERM] * gscale[:, None]).T  # (513, 512), order i,f,o,g
    Wx = np.zeros((128, KC, NG * H), np.float32)
    for k in range(KC):
        kk = 128 if k < 4 else 1
        Wx[0:kk, k, :] = WxT[k * 128:k * 128 + kk, :]
    bGv = ((b_ih + b_hh)[GPERM] * gscale).reshape(NG, H).T     # (128, 4)
    Wh = (2.0 * W_hh[GPERM] * gscale[:, None]).T               # (128, 512)

    w = {
        "Wx": f(Wx),
        "bG": f(bGv),
        "Wh": f(Wh),
        "Wzz": f(np.asarray(inputs["W_zz"]).T),        # (16,128)
        "bzz": f(np.asarray(inputs["b_zz"]).reshape(128, 1)),
        "Wim": f(np.asarray(inputs["W_im"]).T),        # (128,16)
        "Wil": f(np.asarray(inputs["W_il"]).T),
        "Wimh": f(0.5 * np.asarray(inputs["W_im"]).T),
        "Wilh": f(0.5 * np.asarray(inputs["W_il"]).T),
        "IdM": np.eye(128, dtype=ml_dtypes.bfloat16),
        "bim": f(np.asarray(inputs["b_im"]).reshape(ZD, 1)),
        "bil": f(np.asarray(inputs["b_il"]).reshape(ZD, 1)),
        "bil2": f(0.5 * np.asarray(inputs["b_il"]).reshape(ZD, 1)),
        "Wzx1": np.ascontiguousarray(np.asarray(inputs["W_zx1"]).T,
                                     dtype=ml_dtypes.bfloat16),
        "bzx1": f(np.asarray(inputs["b_zx1"]).reshape(128, 1)),
        "Wzx2": np.ascontiguousarray(np.asarray(inputs["W_zx2"]).T,
                                     dtype=ml_dtypes.bfloat16),
        "bzx2": f(np.asarray(inputs["b_zx2"]).reshape(128, 1)),
        "Wgy": np.ascontiguousarray(np.asarray(inputs["W_gy"]).T,
                                    dtype=ml_dtypes.bfloat16),
        "bgy": f(_pack_bgy(np.asarray(inputs["b_gy"], np.float32))),
    }
    for nm in ("g1", "g2", "p1", "p2", "pm", "pl"):
        w["W" + nm] = np.ascontiguousarray(
            np.asarray(inputs["W_" + nm]).T, dtype=ml_dtypes.bfloat16)
        w["b" + nm] = f(np.asarray(inputs["b_" + nm]).reshape(ZD, 1))
    return w


def _pack_bgy(b_gy):
    out = np.zeros((128, KC), np.float32)
    for c in range(KC):
        pc = 128 if c < 4 else 1
        out[0:pc, c] = b_gy[c * 128:c * 128 + pc]
    return out


_CACHE = {}


def _get_compiled(Tn=T, Bs=BS):
    key = (Tn, Bs, PHASES)
    if key in _CACHE:
        return _CACHE[key]
    nc = bacc.Bacc("TRN2", target_bir_lowering=False, debug=False,
                   enable_asserts=False)
    ins = {}

    def din(name, shape, dt=F32):
        ins[name] = nc.dram_tensor(name, list(shape), dt,
                                   kind="ExternalInput").ap()

    din("xb", (Bs, XD, Tn), F32R)
    din("epsT", (ZD, Bs, Tn))
    din("Wx", (128, KC, NG * H), F32R)
    din("bG", (128, NG))
    din("Wh", (128, NG * H))
    din("Wzz", (ZD, H))
    din("bzz", (128, 1))
    din("Wim", (128, ZD))
    din("Wil", (128, ZD))
    din("Wimh", (128, ZD))
    din("Wilh", (128, ZD))
    din("bim", (ZD, 1))
    din("bil", (ZD, 1))
    din("bil2", (ZD, 1))
    for nm in ("g1", "g2", "p1", "p2", "pm", "pl"):
        din("W" + nm, (ZD, ZD), BF16)
        din("b" + nm, (ZD, 1))
    din("Wzx1", (ZD, 128), BF16)
    din("bzx1", (128, 1))
    din("Wzx2", (128, 128), BF16)
    din("bzx2", (128, 1))
    din("Wgy", (128, XD), BF16)
    din("bgy", (128, KC))
    ins["IdM"] = nc.dram_tensor("IdM", [128, 128], BF16,
                                kind="ExternalInput").ap()

    def dout(name, shape):
        return nc.dram_tensor(name, list(shape), F32,
                              kind="ExternalOutput").ap()

    outs = {
        "y": dout("y", (Bs, XD, Tn)),
        "zout": dout("zout", (ZD, Bs, Tn)),
        "zmean": dout("zmean", (ZD, Bs, Tn)),
        "zlogvar": dout("zlogvar", (ZD, Bs, Tn)),
        "zmeanp": dout("zmeanp", (ZD, Bs, Tn)),
        "zlogvarp": dout("zlogvarp", (ZD, Bs, Tn)),
    }

    with tile.TileContext(nc) as tc:
        _build_core_kernel(nc, tc, ins, outs, Tn, Bs)
    nc.compile()
    _CACHE[key] = nc
    return nc


def kernel(**inputs):
    Tn, Bs = T, BS
    nc = _get_compiled(Tn, Bs)
    w = _prep_weights(inputs)

    x = np.ascontiguousarray(np.asarray(inputs["x"], np.float32))
    eps = np.asarray(inputs["eps"], np.float32)   # (T, B, ZD)

    in_maps = []
    for c in range(NCORES):
        b0 = c * Bs
        m = dict(w)
        m["xb"] = np.ascontiguousarray(x[b0:b0 + Bs])
        m["epsT"] = np.ascontiguousarray(
            eps[:, b0:b0 + Bs, :].transpose(2, 1, 0))   # (ZD, Bs, T)
        in_maps.append(m)

    res = bass_utils.run_bass_kernel_spmd(
        nc, in_maps, core_ids=list(range(NCORES)))
    outs = res.results

    y = np.concatenate([outs[c]["y"] for c in range(NCORES)], axis=0)

    def stat(name):  # (ZD,Bs,T) per core -> (T, B, ZD)
        parts = [outs[c][name].transpose(2, 1, 0) for c in range(NCORES)]
        return np.ascontiguousarray(np.concatenate(parts, axis=1),
                                    dtype=np.float32)

    z_mean = stat("zmean")
    z_logvar = stat("zlogvar")
    z_mean_p = stat("zmeanp")
    z_logvar_p = stat("zlogvarp")
    z_out = np.ascontiguousarray(
        np.concatenate([outs[c]["zout"].transpose(1, 0, 2)
                        for c in range(NCORES)], axis=0), dtype=np.float32)
    return (np.ascontiguousarray(y, np.float32), z_mean, z_logvar,
            z_mean_p, z_logvar_p, z_out)
